# revision 25
# baseline (speedup 1.0000x reference)
"""Per-pixel adaptive 5x5 conv (KPN) for Trainium2, 8-core data parallel.

out[g,h,w] = sum_{i,j} core[g,5i+j,h,w] * frames_pad[g,h+i-2,w+j-2]
with g = flattened (B,N) = 16 image planes; 2 planes per NeuronCore.

v4 design — PE-accumulate, DVE-multiply, no GpSimd compute:
  Row layout: partition p owns output rows 4p..4p+3. fin stores the 8-row
  halo (4p-2..4p+5) x 518 padded cols x 2 column-parity copies per
  partition (4.2MB/core vs 10.6MB for the old i-shift replication). The
  parity copy keeps every tap's frame slice 4B-aligned for the DVE fp16
  2x mode: tap (i,j) reads parity j&1 at column offset j+(j&1).
  Per tap: one DVE tensor_mul -> product tile [128, 4*512] (x-aligned),
  then the otherwise-idle TensorEngine accumulates it into PSUM bank r
  (fp32) via identity-stationary matmuls with 4B-aligned moving slices.
  Weights stream as one 2.6MB DMA per 5-tap group (5-deep pipeline);
  identity warmup matmuls run during the DMA ramp. ACT evacuates PSUM
  (fp32->fp16 cast); fin rides the second HWDGE ring (nc.scalar),
  weights/outputs on nc.sync. GpSimd does nothing: its SBUF port contends with DVE
  tensor_tensor ops (the old baseline lost ~45us to that).
"""

import os
import sys

import numpy as np

for _p in ("/opt/trn_rl_repo",):
    if _p not in sys.path and os.path.isdir(_p):
        sys.path.insert(0, _p)

K = 5
NCORES = 8
IMGS_PER_CORE = 2
H = W = 512
RPP = 4            # output rows per partition
FROWS = 8          # stored halo rows per partition
FCOLS = 518
F_FREE = 2 * FROWS * FCOLS   # 8288 (parity, rows, cols)
T_FREE = RPP * W             # 2048 (one tap of weights / products)
WG_FREE = K * T_FREE         # 10240 (one 5-tap group)
O_FREE = RPP * W             # 2048
N_WARMUP = 24

_compiled = {}
last_results = None  # BassKernelResults of the most recent run (for test.py)


def _build_nc():
    import concourse.bacc as bacc
    import concourse.mybir as mybir
    from concourse.masks import make_identity
    from concourse.tile import TileContext

    f16 = mybir.dt.float16
    f32 = mybir.dt.float32

    nc = bacc.Bacc(None, target_bir_lowering=False, debug=False)
    fin = nc.dram_tensor("fin", [IMGS_PER_CORE, 128, F_FREE], f16,
                         kind="ExternalInput")
    win = nc.dram_tensor("win", [IMGS_PER_CORE, 128, K * WG_FREE], f16,
                         kind="ExternalInput")
    oout = nc.dram_tensor("oout", [IMGS_PER_CORE, 128, O_FREE], f16,
                          kind="ExternalOutput")

    with TileContext(nc) as tc:
        with (
            tc.tile_pool(name="const", bufs=1) as cpool,
            tc.tile_pool(name="fpool", bufs=2) as fpool,
            tc.tile_pool(name="wpool", bufs=5) as wpool,
            tc.tile_pool(name="ttap", bufs=6) as ttpool,
            tc.tile_pool(name="opool", bufs=2) as opool,
            tc.tile_pool(name="psum", bufs=2, space="PSUM") as ppool,
        ):
            # fin0's DMA is the first instruction on the ACT ring so the
            # first frames land while the const setup runs elsewhere;
            # fin1 is issued at img1's turn so it does not steal ramp
            # bandwidth from the first weight chunks.
            f_t0 = fpool.tile([128, F_FREE], f16, tag="f")
            HF = F_FREE // 2
            nc.scalar.dma_start(out=f_t0[:, 0:HF], in_=fin[0][:, 0:HF])
            nc.scalar.dma_start(out=f_t0[:, HF:], in_=fin[0][:, HF:])

            ident = cpool.tile([128, 128], f16)
            make_identity(nc, ident[:])

            # PE clock warmup during the DMA ramp: small identity matmuls
            # chained on img1's PSUM tile (never blocks img0's real work).
            pss = []
            for _pi in range(IMGS_PER_CORE):
                ps_lo = ppool.tile([128, O_FREE // 2], f32, tag="pl",
                                   name=f"psl{_pi}")
                ps_hi = ppool.tile([128, O_FREE // 2], f32, tag="ph",
                                   name=f"psh{_pi}")
                pss.append((ps_lo, ps_hi))
            for _ in range(N_WARMUP):
                nc.tensor.matmul(pss[1][1][:, 0:128], ident[:], ident[:],
                                 start=True, stop=True)

            for img in range(IMGS_PER_CORE):
                if img == 0:
                    f_t = f_t0
                else:
                    f_t = fpool.tile([128, F_FREE], f16, tag="f")
                    nc.scalar.dma_start(out=f_t[:], in_=fin[img])
                fv = f_t[:].rearrange("p (par rr y) -> p par rr y",
                                      par=2, rr=FROWS, y=FCOLS)

                ps_lo, ps_hi = pss[img]

                # one 2.6MB weight DMA per 5-tap group (the kernel is
                # bandwidth-bound: finer first-group splits do not help).
                wts = [None] * (K * K)
                for tg in range(K):
                    w_t = wpool.tile([128, WG_FREE], f16)
                    nc.sync.dma_start(
                        out=w_t[:],
                        in_=win[img][:, tg * WG_FREE:(tg + 1) * WG_FREE])
                    for kk in range(K):
                        wts[tg * K + kk] = w_t[:, kk * T_FREE:
                                                (kk + 1) * T_FREE]

                for t in range(K * K):
                    i, j = divmod(t, K)
                    par = j & 1
                    joff = j + par
                    tmp = ttpool.tile([128, T_FREE], f16)
                    nc.vector.tensor_mul(
                        out=tmp[:].rearrange(
                            "p (r c) -> p r c", r=RPP, c=W),
                        in0=wts[t].rearrange(
                            "p (r c) -> p r c", r=RPP, c=W),
                        in1=fv[:, par, i:i + RPP, joff:joff + W])
                    for b in range(RPP):
                        pst = ps_lo if b < 2 else ps_hi
                        nc.tensor.matmul(
                            pst[:, (b % 2) * W:(b % 2 + 1) * W], ident[:],
                            tmp[:, b * W:(b + 1) * W],
                            start=(t == 0), stop=(t == K * K - 1))

                # split evacuation + store halves so the first out DMA
                # overlaps the second ACT copy on the final image.
                o_t = opool.tile([128, O_FREE], f16, tag="o")
                for h, pst in enumerate((ps_lo, ps_hi)):
                    sl = slice(h * O_FREE // 2, (h + 1) * O_FREE // 2)
                    nc.scalar.activation(
                        out=o_t[:, sl], in_=pst[:],
                        func=mybir.ActivationFunctionType.Copy)
                    nc.scalar.dma_start(out=oout[img][:, sl],
                                        in_=o_t[:, sl])
    nc.finalize()
    return nc


def _host_prep(frames, core):
    """Build per-core in_maps. frames [4,4,1,512,512] f32, core [4,4,25,1,512,512]."""
    G = NCORES * IMGS_PER_CORE  # 16
    F = np.ascontiguousarray(frames.reshape(G, H, W))
    Wc = core.reshape(G, K * K, H, W)

    # frames: pad rows 2/2, cols 3/4 -> [G, 516, 519]; halo rows 4p-2+rr
    # (rr 0..7) = padded row 4p+rr; parity copy par starts at col 1-par so
    # tap (i,j) reads parity j&1 at 4B-aligned offset j+(j&1).
    Fp = np.pad(F, ((0, 0), (2, 2), (3, 4))).astype(np.float16)
    rows = 4 * np.arange(128)[:, None] + np.arange(FROWS)[None, :]  # [128,8]
    fprep = np.empty((G, 128, 2, FROWS, FCOLS), np.float16)
    for par in range(2):
        fprep[:, :, par] = Fp[:, rows, (1 - par):(1 - par) + FCOLS]

    # weights: [g, t, 4p+r, c] -> [g, p, tg, k, r, c] (all groups
    # contiguous per partition row so chunked DMAs slice cleanly)
    w16 = Wc.astype(np.float16)
    wprep = np.ascontiguousarray(
        w16.reshape(G, K, K, 128, RPP, W).transpose(0, 3, 1, 2, 4, 5))

    in_maps = []
    for c in range(NCORES):
        g0 = c * IMGS_PER_CORE
        in_maps.append({
            "fin": np.ascontiguousarray(
                fprep[g0:g0 + IMGS_PER_CORE].reshape(
                    IMGS_PER_CORE, 128, F_FREE)),
            "win": np.ascontiguousarray(
                wprep[g0:g0 + IMGS_PER_CORE].reshape(
                    IMGS_PER_CORE, 128, K * WG_FREE)),
        })
    return in_maps


def kernel(frames, core, bias):
    global last_results
    from concourse.bass_utils import run_bass_kernel_spmd

    frames = np.asarray(frames, dtype=np.float32)
    core = np.asarray(core, dtype=np.float32)

    if "nc" not in _compiled:
        _compiled["nc"] = _build_nc()
    nc = _compiled["nc"]

    in_maps = _host_prep(frames, core)
    trace = os.environ.get("KC_TRACE") == "1"
    tmpdir = os.environ.get("KC_TRACE_DIR") or None
    if tmpdir:
        os.makedirs(tmpdir, exist_ok=True)
    res = run_bass_kernel_spmd(nc, in_maps, list(range(NCORES)), trace=trace,
                               tmpdir=tmpdir)
    last_results = res

    G = NCORES * IMGS_PER_CORE
    out = np.empty((G, H, W), np.float32)
    for c in range(NCORES):
        o = res.results[c]["oout"]  # [2, 128, 2048] f16
        for img in range(IMGS_PER_CORE):
            out[c * IMGS_PER_CORE + img] = (
                o[img].astype(np.float32).reshape(H, W))
    return out.reshape(4, 4, H, W)


# revision 27
# speedup vs baseline: 1.0200x; 1.0200x over previous
"""Per-pixel adaptive 5x5 conv (KPN) for Trainium2, 8-core data parallel.

out[g,h,w] = sum_{i,j} core[g,5i+j,h,w] * frames_pad[g,h+i-2,w+j-2]
with g = flattened (B,N) = 16 image planes; 2 planes per NeuronCore.

v4 design — PE-accumulate, DVE-multiply, no GpSimd compute:
  Row layout: partition p owns output rows 4p..4p+3. fin stores the 8-row
  halo (4p-2..4p+5) x 518 padded cols x 2 column-parity copies per
  partition (4.2MB/core vs 10.6MB for the old i-shift replication). The
  parity copy keeps every tap's frame slice 4B-aligned for the DVE fp16
  2x mode: tap (i,j) reads parity j&1 at column offset j+(j&1).
  Per tap: one DVE tensor_mul -> product tile [128, 4*512] (x-aligned),
  then the otherwise-idle TensorEngine accumulates it into PSUM bank r
  (fp32) via identity-stationary matmuls with 4B-aligned moving slices.
  Weights stream as one 2.6MB DMA per 5-tap group (5-deep pipeline);
  identity warmup matmuls run during the DMA ramp. PSUM is split into
  lo/hi 2-bank tiles per image so the ACT evacuation (fp32->fp16 cast)
  overlaps the last matmuls. fin/out ride the second HWDGE ring
  (nc.scalar) so output stores never head-of-line-block the weight
  dispatch queue (nc.sync). GpSimd does nothing: its SBUF port contends
  with DVE tensor_tensor ops (the old baseline lost ~45us to that).
"""

import os
import sys

import numpy as np

for _p in ("/opt/trn_rl_repo",):
    if _p not in sys.path and os.path.isdir(_p):
        sys.path.insert(0, _p)

K = 5
NCORES = 8
IMGS_PER_CORE = 2
H = W = 512
RPP = 4            # output rows per partition
FROWS = 8          # stored halo rows per partition
FCOLS = 518
F_FREE = 2 * FROWS * FCOLS   # 8288 (parity, rows, cols)
T_FREE = RPP * W             # 2048 (one tap of weights / products)
WG_FREE = K * T_FREE         # 10240 (one 5-tap group)
O_FREE = RPP * W             # 2048
N_WARMUP = 16

_compiled = {}
last_results = None  # BassKernelResults of the most recent run (for test.py)


def _build_nc():
    import concourse.bacc as bacc
    import concourse.mybir as mybir
    from concourse.masks import make_identity
    from concourse.tile import TileContext

    f16 = mybir.dt.float16
    f32 = mybir.dt.float32

    nc = bacc.Bacc(None, target_bir_lowering=False, debug=False)
    fin = nc.dram_tensor("fin", [IMGS_PER_CORE, 128, F_FREE], f16,
                         kind="ExternalInput")
    win = nc.dram_tensor("win", [IMGS_PER_CORE, 128, K * WG_FREE], f16,
                         kind="ExternalInput")
    oout = nc.dram_tensor("oout", [IMGS_PER_CORE, 128, O_FREE], f16,
                          kind="ExternalOutput")

    with TileContext(nc) as tc:
        with (
            tc.tile_pool(name="const", bufs=1) as cpool,
            tc.tile_pool(name="fpool", bufs=2) as fpool,
            tc.tile_pool(name="wpool", bufs=5) as wpool,
            tc.tile_pool(name="ttap", bufs=6) as ttpool,
            tc.tile_pool(name="opool", bufs=2) as opool,
            tc.tile_pool(name="psum", bufs=2, space="PSUM") as ppool,
        ):
            # fin0's DMA is the first instruction on the ACT ring so the
            # first frames land while the const setup runs elsewhere;
            # fin1 is issued at img1's turn so it does not steal ramp
            # bandwidth from the first weight chunks.
            f_t0 = fpool.tile([128, F_FREE], f16, tag="f")
            HF = F_FREE // 2
            nc.scalar.dma_start(out=f_t0[:, 0:HF], in_=fin[0][:, 0:HF])
            nc.scalar.dma_start(out=f_t0[:, HF:], in_=fin[0][:, HF:])

            ident = cpool.tile([128, 128], f16)
            make_identity(nc, ident[:])

            # PE clock warmup during the DMA ramp: small identity matmuls
            # chained on img1's PSUM tile (never blocks img0's real work).
            pss = []
            for _pi in range(IMGS_PER_CORE):
                ps_lo = ppool.tile([128, O_FREE // 2], f32, tag="pl",
                                   name=f"psl{_pi}")
                ps_hi = ppool.tile([128, O_FREE // 2], f32, tag="ph",
                                   name=f"psh{_pi}")
                pss.append((ps_lo, ps_hi))
            # Warmup reads fin0 (lands ~13us) so the PE busy window runs
            # right up to the first real matmul (~21us) — an earlier
            # warmup lets the HAM clock throttle re-arm during the idle
            # gap before real work.
            for _ in range(N_WARMUP):
                nc.tensor.matmul(pss[1][1][:, 0:512], ident[:],
                                 f_t0[:, 0:512], start=True, stop=True)

            for img in range(IMGS_PER_CORE):
                if img == 0:
                    f_t = f_t0
                else:
                    f_t = fpool.tile([128, F_FREE], f16, tag="f")
                    nc.scalar.dma_start(out=f_t[:], in_=fin[img])
                fv = f_t[:].rearrange("p (par rr y) -> p par rr y",
                                      par=2, rr=FROWS, y=FCOLS)

                ps_lo, ps_hi = pss[img]

                # one 2.6MB weight DMA per 5-tap group (the kernel is
                # bandwidth-bound: finer first-group splits do not help).
                wts = [None] * (K * K)
                for tg in range(K):
                    w_t = wpool.tile([128, WG_FREE], f16)
                    nc.sync.dma_start(
                        out=w_t[:],
                        in_=win[img][:, tg * WG_FREE:(tg + 1) * WG_FREE])
                    for kk in range(K):
                        wts[tg * K + kk] = w_t[:, kk * T_FREE:
                                                (kk + 1) * T_FREE]

                for t in range(K * K):
                    i, j = divmod(t, K)
                    par = j & 1
                    joff = j + par
                    tmp = ttpool.tile([128, T_FREE], f16)
                    nc.vector.tensor_mul(
                        out=tmp[:].rearrange(
                            "p (r c) -> p r c", r=RPP, c=W),
                        in0=wts[t].rearrange(
                            "p (r c) -> p r c", r=RPP, c=W),
                        in1=fv[:, par, i:i + RPP, joff:joff + W])
                    for b in range(RPP):
                        pst = ps_lo if b < 2 else ps_hi
                        nc.tensor.matmul(
                            pst[:, (b % 2) * W:(b % 2 + 1) * W], ident[:],
                            tmp[:, b * W:(b + 1) * W],
                            start=(t == 0), stop=(t == K * K - 1))

                # split evacuation + store halves so the first out DMA
                # overlaps the second ACT copy on the final image.
                o_t = opool.tile([128, O_FREE], f16, tag="o")
                for h, pst in enumerate((ps_lo, ps_hi)):
                    sl = slice(h * O_FREE // 2, (h + 1) * O_FREE // 2)
                    nc.scalar.activation(
                        out=o_t[:, sl], in_=pst[:],
                        func=mybir.ActivationFunctionType.Copy)
                    nc.scalar.dma_start(out=oout[img][:, sl],
                                        in_=o_t[:, sl])
    nc.finalize()
    return nc


def _host_prep(frames, core):
    """Build per-core in_maps. frames [4,4,1,512,512] f32, core [4,4,25,1,512,512]."""
    G = NCORES * IMGS_PER_CORE  # 16
    F = np.ascontiguousarray(frames.reshape(G, H, W))
    Wc = core.reshape(G, K * K, H, W)

    # frames: pad rows 2/2, cols 3/4 -> [G, 516, 519]; halo rows 4p-2+rr
    # (rr 0..7) = padded row 4p+rr; parity copy par starts at col 1-par so
    # tap (i,j) reads parity j&1 at 4B-aligned offset j+(j&1).
    Fp = np.pad(F, ((0, 0), (2, 2), (3, 4))).astype(np.float16)
    rows = 4 * np.arange(128)[:, None] + np.arange(FROWS)[None, :]  # [128,8]
    fprep = np.empty((G, 128, 2, FROWS, FCOLS), np.float16)
    for par in range(2):
        fprep[:, :, par] = Fp[:, rows, (1 - par):(1 - par) + FCOLS]

    # weights: [g, t, 4p+r, c] -> [g, p, tg, k, r, c] (all groups
    # contiguous per partition row so chunked DMAs slice cleanly)
    w16 = Wc.astype(np.float16)
    wprep = np.ascontiguousarray(
        w16.reshape(G, K, K, 128, RPP, W).transpose(0, 3, 1, 2, 4, 5))

    in_maps = []
    for c in range(NCORES):
        g0 = c * IMGS_PER_CORE
        in_maps.append({
            "fin": np.ascontiguousarray(
                fprep[g0:g0 + IMGS_PER_CORE].reshape(
                    IMGS_PER_CORE, 128, F_FREE)),
            "win": np.ascontiguousarray(
                wprep[g0:g0 + IMGS_PER_CORE].reshape(
                    IMGS_PER_CORE, 128, K * WG_FREE)),
        })
    return in_maps


def kernel(frames, core, bias):
    global last_results
    from concourse.bass_utils import run_bass_kernel_spmd

    frames = np.asarray(frames, dtype=np.float32)
    core = np.asarray(core, dtype=np.float32)

    if "nc" not in _compiled:
        _compiled["nc"] = _build_nc()
    nc = _compiled["nc"]

    in_maps = _host_prep(frames, core)
    trace = os.environ.get("KC_TRACE") == "1"
    tmpdir = os.environ.get("KC_TRACE_DIR") or None
    if tmpdir:
        os.makedirs(tmpdir, exist_ok=True)
    res = run_bass_kernel_spmd(nc, in_maps, list(range(NCORES)), trace=trace,
                               tmpdir=tmpdir)
    last_results = res

    G = NCORES * IMGS_PER_CORE
    out = np.empty((G, H, W), np.float32)
    for c in range(NCORES):
        o = res.results[c]["oout"]  # [2, 128, 2048] f16
        for img in range(IMGS_PER_CORE):
            out[c * IMGS_PER_CORE + img] = (
                o[img].astype(np.float32).reshape(H, W))
    return out.reshape(4, 4, H, W)


# revision 28
# speedup vs baseline: 1.0829x; 1.0616x over previous
"""Per-pixel adaptive 5x5 conv (KPN) for Trainium2, 8-core data parallel.

out[g,h,w] = sum_{i,j} core[g,5i+j,h,w] * frames_pad[g,h+i-2,w+j-2]
with g = flattened (B,N) = 16 image planes; 2 planes per NeuronCore.

v4 design — PE-accumulate, DVE-multiply, no GpSimd compute:
  Row layout: partition p owns output rows 4p..4p+3. fin stores the 8-row
  halo (4p-2..4p+5) x 518 padded cols x 2 column-parity copies per
  partition (4.2MB/core vs 10.6MB for the old i-shift replication). The
  parity copy keeps every tap's frame slice 4B-aligned for the DVE fp16
  2x mode: tap (i,j) reads parity j&1 at column offset j+(j&1).
  Per tap: one DVE tensor_mul -> product tile [128, 4*512] (x-aligned),
  then the otherwise-idle TensorEngine accumulates it into PSUM bank r
  (fp32) via identity-stationary matmuls with 4B-aligned moving slices.
  Weights stream as one 2.6MB DMA per 5-tap group (5-deep pipeline);
  identity warmup matmuls run during the DMA ramp. PSUM is split into
  lo/hi 2-bank tiles per image so the ACT evacuation (fp32->fp16 cast)
  overlaps the last matmuls. fin/out ride the second HWDGE ring
  (nc.scalar) so output stores never head-of-line-block the weight
  dispatch queue (nc.sync). GpSimd does nothing: its SBUF port contends
  with DVE tensor_tensor ops (the old baseline lost ~45us to that).
"""

import os
import sys

import numpy as np

for _p in ("/opt/trn_rl_repo",):
    if _p not in sys.path and os.path.isdir(_p):
        sys.path.insert(0, _p)

K = 5
NCORES = 8
IMGS_PER_CORE = 2
H = W = 512
RPP = 4            # output rows per partition
FROWS = 8          # stored halo rows per partition
FCOLS = 518
F_FREE = 2 * FROWS * FCOLS   # 8288 (parity, rows, cols)
T_FREE = RPP * W             # 2048 (one tap of weights / products)
WG_FREE = K * T_FREE         # 10240 (one 5-tap group)
O_FREE = RPP * W             # 2048
N_WARMUP = 16

_compiled = {}
last_results = None  # BassKernelResults of the most recent run (for test.py)


def _build_nc():
    import concourse.bacc as bacc
    import concourse.mybir as mybir
    from concourse.masks import make_identity
    from concourse.tile import TileContext

    f16 = mybir.dt.float16
    f32 = mybir.dt.float32

    nc = bacc.Bacc(None, target_bir_lowering=False, debug=False)
    fin = nc.dram_tensor("fin", [IMGS_PER_CORE, 128, F_FREE], f16,
                         kind="ExternalInput")
    win = nc.dram_tensor("win", [IMGS_PER_CORE, 128, K * WG_FREE], f16,
                         kind="ExternalInput")
    oout = nc.dram_tensor("oout", [IMGS_PER_CORE, 128, O_FREE], f16,
                          kind="ExternalOutput")

    with TileContext(nc) as tc:
        with (
            tc.tile_pool(name="const", bufs=1) as cpool,
            tc.tile_pool(name="fpool", bufs=2) as fpool,
            tc.tile_pool(name="wpool", bufs=5) as wpool,
            tc.tile_pool(name="ttap", bufs=6) as ttpool,
            tc.tile_pool(name="opool", bufs=2) as opool,
            tc.tile_pool(name="psum", bufs=2, space="PSUM") as ppool,
        ):
            # fin0's DMA is the first instruction on the ACT ring so the
            # first frames land while the const setup runs elsewhere;
            # fin1 is issued at img1's turn so it does not steal ramp
            # bandwidth from the first weight chunks.
            f_t0 = fpool.tile([128, F_FREE], f16, tag="f")
            HF = F_FREE // 2
            nc.scalar.dma_start(out=f_t0[:, 0:HF], in_=fin[0][:, 0:HF])
            nc.scalar.dma_start(out=f_t0[:, HF:], in_=fin[0][:, HF:])

            ident = cpool.tile([128, 128], f16)
            make_identity(nc, ident[:])

            # PE clock warmup during the DMA ramp: small identity matmuls
            # chained on img1's PSUM tile (never blocks img0's real work).
            pss = []
            for _pi in range(IMGS_PER_CORE):
                ps_lo = ppool.tile([128, O_FREE // 2], f32, tag="pl",
                                   name=f"psl{_pi}")
                ps_hi = ppool.tile([128, O_FREE // 2], f32, tag="ph",
                                   name=f"psh{_pi}")
                pss.append((ps_lo, ps_hi))
            # Warmup reads fin0 (lands ~13us) so the PE busy window runs
            # right up to the first real matmul (~21us) — an earlier
            # warmup lets the HAM clock throttle re-arm during the idle
            # gap before real work.
            for _ in range(N_WARMUP):
                nc.tensor.matmul(pss[1][1][:, 0:512], ident[:],
                                 f_t0[:, 0:512], start=True, stop=True)

            for img in range(IMGS_PER_CORE):
                if img == 0:
                    f_t = f_t0
                else:
                    f_t = fpool.tile([128, F_FREE], f16, tag="f")
                    nc.scalar.dma_start(out=f_t[:], in_=fin[img])
                fv = f_t[:].rearrange("p (par rr y) -> p par rr y",
                                      par=2, rr=FROWS, y=FCOLS)

                ps_lo, ps_hi = pss[img]

                # one 2.6MB weight DMA per 5-tap group (the kernel is
                # bandwidth-bound: finer first-group splits do not help).
                wts = [None] * (K * K)
                for tg in range(K):
                    w_t = wpool.tile([128, WG_FREE], f16)
                    nc.sync.dma_start(
                        out=w_t[:],
                        in_=win[img][:, tg * WG_FREE:(tg + 1) * WG_FREE])
                    for kk in range(K):
                        wts[tg * K + kk] = w_t[:, kk * T_FREE:
                                                (kk + 1) * T_FREE]

                for t in range(K * K):
                    i, j = divmod(t, K)
                    par = j & 1
                    joff = j + par
                    tmp = ttpool.tile([128, T_FREE], f16)
                    nc.vector.tensor_mul(
                        out=tmp[:].rearrange(
                            "p (r c) -> p r c", r=RPP, c=W),
                        in0=wts[t].rearrange(
                            "p (r c) -> p r c", r=RPP, c=W),
                        in1=fv[:, par, i:i + RPP, joff:joff + W])
                    for b in range(RPP):
                        pst = ps_lo if b < 2 else ps_hi
                        nc.tensor.matmul(
                            pst[:, (b % 2) * W:(b % 2 + 1) * W], ident[:],
                            tmp[:, b * W:(b + 1) * W],
                            start=(t == 0), stop=(t == K * K - 1))
                    if t % K == K - 1 and t != K * K - 1:
                        # group boundary: DVE stalls ~2-3us on the next
                        # weight DMA; idle >1.7us re-arms the PE HAM clock
                        # throttle and halves matmul speed for the rest of
                        # the image. ldweights of the (unchanged) identity
                        # keeps the PE busy with no PSUM/SBUF side effects.
                        for _ in range(24):
                            nc.tensor.ldweights(ident[:])

                # split evacuation + store halves so the first out DMA
                # overlaps the second ACT copy on the final image.
                o_t = opool.tile([128, O_FREE], f16, tag="o")
                out_eng = nc.scalar if img == 0 else nc.sync
                for h, pst in enumerate((ps_lo, ps_hi)):
                    sl = slice(h * O_FREE // 2, (h + 1) * O_FREE // 2)
                    nc.scalar.activation(
                        out=o_t[:, sl], in_=pst[:],
                        func=mybir.ActivationFunctionType.Copy)
                    out_eng.dma_start(out=oout[img][:, sl],
                                      in_=o_t[:, sl])
    nc.finalize()
    return nc


def _host_prep(frames, core):
    """Build per-core in_maps. frames [4,4,1,512,512] f32, core [4,4,25,1,512,512]."""
    G = NCORES * IMGS_PER_CORE  # 16
    F = np.ascontiguousarray(frames.reshape(G, H, W))
    Wc = core.reshape(G, K * K, H, W)

    # frames: pad rows 2/2, cols 3/4 -> [G, 516, 519]; halo rows 4p-2+rr
    # (rr 0..7) = padded row 4p+rr; parity copy par starts at col 1-par so
    # tap (i,j) reads parity j&1 at 4B-aligned offset j+(j&1).
    Fp = np.pad(F, ((0, 0), (2, 2), (3, 4))).astype(np.float16)
    rows = 4 * np.arange(128)[:, None] + np.arange(FROWS)[None, :]  # [128,8]
    fprep = np.empty((G, 128, 2, FROWS, FCOLS), np.float16)
    for par in range(2):
        fprep[:, :, par] = Fp[:, rows, (1 - par):(1 - par) + FCOLS]

    # weights: [g, t, 4p+r, c] -> [g, p, tg, k, r, c] (all groups
    # contiguous per partition row so chunked DMAs slice cleanly)
    w16 = Wc.astype(np.float16)
    wprep = np.ascontiguousarray(
        w16.reshape(G, K, K, 128, RPP, W).transpose(0, 3, 1, 2, 4, 5))

    in_maps = []
    for c in range(NCORES):
        g0 = c * IMGS_PER_CORE
        in_maps.append({
            "fin": np.ascontiguousarray(
                fprep[g0:g0 + IMGS_PER_CORE].reshape(
                    IMGS_PER_CORE, 128, F_FREE)),
            "win": np.ascontiguousarray(
                wprep[g0:g0 + IMGS_PER_CORE].reshape(
                    IMGS_PER_CORE, 128, K * WG_FREE)),
        })
    return in_maps


def kernel(frames, core, bias):
    global last_results
    from concourse.bass_utils import run_bass_kernel_spmd

    frames = np.asarray(frames, dtype=np.float32)
    core = np.asarray(core, dtype=np.float32)

    if "nc" not in _compiled:
        _compiled["nc"] = _build_nc()
    nc = _compiled["nc"]

    in_maps = _host_prep(frames, core)
    trace = os.environ.get("KC_TRACE") == "1"
    tmpdir = os.environ.get("KC_TRACE_DIR") or None
    if tmpdir:
        os.makedirs(tmpdir, exist_ok=True)
    res = run_bass_kernel_spmd(nc, in_maps, list(range(NCORES)), trace=trace,
                               tmpdir=tmpdir)
    last_results = res

    G = NCORES * IMGS_PER_CORE
    out = np.empty((G, H, W), np.float32)
    for c in range(NCORES):
        o = res.results[c]["oout"]  # [2, 128, 2048] f16
        for img in range(IMGS_PER_CORE):
            out[c * IMGS_PER_CORE + img] = (
                o[img].astype(np.float32).reshape(H, W))
    return out.reshape(4, 4, H, W)


# revision 30
# speedup vs baseline: 1.1980x; 1.1063x over previous
"""Per-pixel adaptive 5x5 conv (KPN) for Trainium2, 8-core data parallel.

out[g,h,w] = sum_{i,j} core[g,5i+j,h,w] * frames_pad[g,h+i-2,w+j-2]
with g = flattened (B,N) = 16 image planes; 2 planes per NeuronCore.

v4 design — PE-accumulate, DVE-multiply, no GpSimd compute:
  Row layout: partition p owns output rows 4p..4p+3. fin stores the 8-row
  halo (4p-2..4p+5) x 518 padded cols x 2 column-parity copies per
  partition (4.2MB/core vs 10.6MB for the old i-shift replication). The
  parity copy keeps every tap's frame slice 4B-aligned for the DVE fp16
  2x mode: tap (i,j) reads parity j&1 at column offset j+(j&1).
  Per tap: one DVE tensor_mul -> product tile [128, 4*512] (x-aligned),
  then the otherwise-idle TensorEngine accumulates it into PSUM bank r
  (fp32) via identity-stationary matmuls with 4B-aligned moving slices.
  Weights stream as one 2.6MB DMA per 5-tap group (5-deep pipeline);
  identity warmup matmuls run during the DMA ramp. PSUM is split into
  lo/hi 2-bank tiles per image so the ACT evacuation (fp32->fp16 cast)
  overlaps the last matmuls. fin/out ride the second HWDGE ring
  (nc.scalar) so output stores never head-of-line-block the weight
  dispatch queue (nc.sync). GpSimd does nothing: its SBUF port contends
  with DVE tensor_tensor ops (the old baseline lost ~45us to that).
"""

import os
import sys

import numpy as np

for _p in ("/opt/trn_rl_repo",):
    if _p not in sys.path and os.path.isdir(_p):
        sys.path.insert(0, _p)

K = 5
NCORES = 8
IMGS_PER_CORE = 2
H = W = 512
RPP = 4            # output rows per partition
FROWS = 8          # stored halo rows per partition
FCOLS = 518
F_FREE = 2 * FROWS * FCOLS   # 8288 (parity, rows, cols)
T_FREE = RPP * W             # 2048 (one tap of weights / products)
WG_FREE = K * T_FREE         # 10240 (one 5-tap group)
O_FREE = RPP * W             # 2048
N_WARMUP = 16

_compiled = {}
last_results = None  # BassKernelResults of the most recent run (for test.py)


def _build_nc():
    import concourse.bacc as bacc
    import concourse.mybir as mybir
    from concourse.masks import make_identity
    from concourse.tile import TileContext

    f16 = mybir.dt.float16
    f32 = mybir.dt.float32

    nc = bacc.Bacc(None, target_bir_lowering=False, debug=False)
    fin = nc.dram_tensor("fin", [IMGS_PER_CORE, 128, F_FREE], f16,
                         kind="ExternalInput")
    win = nc.dram_tensor("win", [IMGS_PER_CORE, 128, K * WG_FREE], f16,
                         kind="ExternalInput")
    oout = nc.dram_tensor("oout", [IMGS_PER_CORE, 128, O_FREE], f16,
                          kind="ExternalOutput")

    with TileContext(nc) as tc:
        with (
            tc.tile_pool(name="const", bufs=1) as cpool,
            tc.tile_pool(name="fpool", bufs=2) as fpool,
            tc.tile_pool(name="wpool", bufs=5) as wpool,
            tc.tile_pool(name="ttap", bufs=6) as ttpool,
            tc.tile_pool(name="opool", bufs=2) as opool,
            tc.tile_pool(name="psum", bufs=2, space="PSUM") as ppool,
        ):
            # fin0's DMA is the first instruction on the ACT ring so the
            # first frames land while the const setup runs elsewhere;
            # fin1 is issued at img1's turn so it does not steal ramp
            # bandwidth from the first weight chunks.
            f_t0 = fpool.tile([128, F_FREE], f16, tag="f")
            HF = F_FREE // 2
            nc.scalar.dma_start(out=f_t0[:, 0:HF], in_=fin[0][:, 0:HF])
            nc.scalar.dma_start(out=f_t0[:, HF:], in_=fin[0][:, HF:])

            ident = cpool.tile([128, 128], f16)
            make_identity(nc, ident[:])

            # PE clock warmup during the DMA ramp: small identity matmuls
            # chained on img1's PSUM tile (never blocks img0's real work).
            pss = []
            for _pi in range(IMGS_PER_CORE):
                ps_lo = ppool.tile([128, O_FREE // 2], f32, tag="pl",
                                   name=f"psl{_pi}")
                ps_hi = ppool.tile([128, O_FREE // 2], f32, tag="ph",
                                   name=f"psh{_pi}")
                pss.append((ps_lo, ps_hi))
            # Warmup reads fin0 (lands ~13us) so the PE busy window runs
            # right up to the first real matmul (~21us) — an earlier
            # warmup lets the HAM clock throttle re-arm during the idle
            # gap before real work.
            for _ in range(N_WARMUP):
                nc.tensor.matmul(pss[1][1][:, 0:512], ident[:],
                                 f_t0[:, 0:512], start=True, stop=True)

            for img in range(IMGS_PER_CORE):
                if img == 0:
                    f_t = f_t0
                else:
                    # re-warm the PE across the image-boundary DVE stall
                    # (~2.8us > the HAM throttle's ~1.7us idle window);
                    # same reset-by-start=True pattern as the initial
                    # warmup, so img1's real accumulation is untouched.
                    for _ in range(6):
                        nc.tensor.matmul(pss[1][1][:, 0:512], ident[:],
                                         f_t0[:, 0:512],
                                         start=True, stop=True)
                    f_t = fpool.tile([128, F_FREE], f16, tag="f")
                    nc.scalar.dma_start(out=f_t[:], in_=fin[img])
                fv = f_t[:].rearrange("p (par rr y) -> p par rr y",
                                      par=2, rr=FROWS, y=FCOLS)

                ps_lo, ps_hi = pss[img]

                # one 2.6MB weight DMA per 5-tap group (the kernel is
                # bandwidth-bound: finer first-group splits do not help).
                wts = [None] * (K * K)
                for tg in range(K):
                    w_t = wpool.tile([128, WG_FREE], f16)
                    nc.sync.dma_start(
                        out=w_t[:],
                        in_=win[img][:, tg * WG_FREE:(tg + 1) * WG_FREE])
                    for kk in range(K):
                        wts[tg * K + kk] = w_t[:, kk * T_FREE:
                                                (kk + 1) * T_FREE]

                for t in range(K * K):
                    i, j = divmod(t, K)
                    par = j & 1
                    joff = j + par
                    tmp = ttpool.tile([128, T_FREE], f16)
                    nc.vector.tensor_mul(
                        out=tmp[:].rearrange(
                            "p (r c) -> p r c", r=RPP, c=W),
                        in0=wts[t].rearrange(
                            "p (r c) -> p r c", r=RPP, c=W),
                        in1=fv[:, par, i:i + RPP, joff:joff + W])
                    for b in range(RPP):
                        pst = ps_lo if b < 2 else ps_hi
                        nc.tensor.matmul(
                            pst[:, (b % 2) * W:(b % 2 + 1) * W], ident[:],
                            tmp[:, b * W:(b + 1) * W],
                            start=(t == 0), stop=(t == K * K - 1))

                # split evacuation + store halves so the first out DMA
                # overlaps the second ACT copy on the final image.
                o_t = opool.tile([128, O_FREE], f16, tag="o")
                for h, pst in enumerate((ps_lo, ps_hi)):
                    sl = slice(h * O_FREE // 2, (h + 1) * O_FREE // 2)
                    nc.scalar.activation(
                        out=o_t[:, sl], in_=pst[:],
                        func=mybir.ActivationFunctionType.Copy)
                    nc.scalar.dma_start(out=oout[img][:, sl],
                                        in_=o_t[:, sl])
    nc.finalize()
    return nc


def _host_prep(frames, core):
    """Build per-core in_maps. frames [4,4,1,512,512] f32, core [4,4,25,1,512,512]."""
    G = NCORES * IMGS_PER_CORE  # 16
    F = np.ascontiguousarray(frames.reshape(G, H, W))
    Wc = core.reshape(G, K * K, H, W)

    # frames: pad rows 2/2, cols 3/4 -> [G, 516, 519]; halo rows 4p-2+rr
    # (rr 0..7) = padded row 4p+rr; parity copy par starts at col 1-par so
    # tap (i,j) reads parity j&1 at 4B-aligned offset j+(j&1).
    Fp = np.pad(F, ((0, 0), (2, 2), (3, 4))).astype(np.float16)
    rows = 4 * np.arange(128)[:, None] + np.arange(FROWS)[None, :]  # [128,8]
    fprep = np.empty((G, 128, 2, FROWS, FCOLS), np.float16)
    for par in range(2):
        fprep[:, :, par] = Fp[:, rows, (1 - par):(1 - par) + FCOLS]

    # weights: [g, t, 4p+r, c] -> [g, p, tg, k, r, c] (all groups
    # contiguous per partition row so chunked DMAs slice cleanly)
    w16 = Wc.astype(np.float16)
    wprep = np.ascontiguousarray(
        w16.reshape(G, K, K, 128, RPP, W).transpose(0, 3, 1, 2, 4, 5))

    in_maps = []
    for c in range(NCORES):
        g0 = c * IMGS_PER_CORE
        in_maps.append({
            "fin": np.ascontiguousarray(
                fprep[g0:g0 + IMGS_PER_CORE].reshape(
                    IMGS_PER_CORE, 128, F_FREE)),
            "win": np.ascontiguousarray(
                wprep[g0:g0 + IMGS_PER_CORE].reshape(
                    IMGS_PER_CORE, 128, K * WG_FREE)),
        })
    return in_maps


def kernel(frames, core, bias):
    global last_results
    from concourse.bass_utils import run_bass_kernel_spmd

    frames = np.asarray(frames, dtype=np.float32)
    core = np.asarray(core, dtype=np.float32)

    if "nc" not in _compiled:
        _compiled["nc"] = _build_nc()
    nc = _compiled["nc"]

    in_maps = _host_prep(frames, core)
    trace = os.environ.get("KC_TRACE") == "1"
    tmpdir = os.environ.get("KC_TRACE_DIR") or None
    if tmpdir:
        os.makedirs(tmpdir, exist_ok=True)
    res = run_bass_kernel_spmd(nc, in_maps, list(range(NCORES)), trace=trace,
                               tmpdir=tmpdir)
    last_results = res

    G = NCORES * IMGS_PER_CORE
    out = np.empty((G, H, W), np.float32)
    for c in range(NCORES):
        o = res.results[c]["oout"]  # [2, 128, 2048] f16
        for img in range(IMGS_PER_CORE):
            out[c * IMGS_PER_CORE + img] = (
                o[img].astype(np.float32).reshape(H, W))
    return out.reshape(4, 4, H, W)
